# revision 3
# baseline (speedup 1.0000x reference)
"""Trainium2 Bass kernel for nn_BioEncoder (GCN + 3 MLP branches), 8 cores.

Sharding: nodes/edges by dst block across 8 cores (graph-parallel); the small
MLP branches are replicated (full batch) on every core; weights replicated.

GCN aggregation: edges sorted by dst window (128), per-tile indirect-DMA row
gather (128 rows/call, one per partition) + iota/tensor_scalar selection
matrix + PE matmul scatter-add into PSUM (feature-major), giving
out^T = sum_e norm_e * h[src_e] per dst window.  Symmetric normalization is
folded into the per-edge S-matrix weights (norm = dinv[src]*dinv[dst]).
BatchNorm batch stats via bn_stats/bn_aggr + AllReduce; h1 table AllGathered
for layer-2 gathers.
"""

import numpy as np

import concourse.bacc as bacc
import concourse.bass as bass
import concourse.mybir as mybir
import concourse.tile as tile
from contextlib import ExitStack
from concourse._compat import cdiv, get_trn_type
from concourse.bass_utils import run_bass_kernel_spmd

P = 128
NRANKS = 8
f32 = mybir.dt.float32
i32 = mybir.dt.int32
AF = mybir.ActivationFunctionType
ALU = mybir.AluOpType
EPS = 1e-5


# ---------------------------------------------------------------- host prep
def _build_plan(src_g, dst_g, norm_g, nb):
    """Global (self-loop-augmented) edges -> per-core packed tile streams with
    a schedule (tile->window map) UNIFORM across cores (SPMD: one program).

    Returns (eidx [8,128,T], edst [8,128,T], enrm [8,128,T], tile_win [T])."""
    nw = cdiv(nb, P)
    core = dst_g // nb
    dloc = dst_g - core * nb
    win = dloc // P
    # per (core, window) edge lists
    counts = np.zeros((NRANKS, nw), np.int64)
    np.add.at(counts, (core, win), 1)
    tiles_w = np.maximum(1, -(-counts.max(axis=0) // P))  # global per-window tiles
    T = int(tiles_w.sum())

    eidx = np.zeros((NRANKS, T * P), np.int32)
    edst = -np.ones((NRANKS, T * P), np.float32)
    enrm = np.zeros((NRANKS, T * P), np.float32)
    tile_win = np.repeat(np.arange(nw), tiles_w)

    # window start offset (in padded slots) per window
    wstart = np.concatenate([[0], np.cumsum(tiles_w)])[:-1] * P

    order = np.lexsort((win, core))
    s_s, d_s, n_s, c_s, w_s = (
        src_g[order],
        (dloc - win * P)[order],
        norm_g[order],
        core[order],
        win[order],
    )
    # position within (core, window) group
    grp = c_s * nw + w_s
    first = np.ones(len(grp), bool)
    first[1:] = grp[1:] != grp[:-1]
    gstart = np.where(first)[0]
    gid = np.cumsum(first) - 1
    pos_in_grp = np.arange(len(grp)) - gstart[gid]
    slot = wstart[w_s] + pos_in_grp
    eidx[c_s, slot] = s_s
    edst[c_s, slot] = d_s
    enrm[c_s, slot] = n_s

    def pack(a):
        # slot i -> tile i//128, partition i%128 ; SBUF layout [128, T]
        return np.ascontiguousarray(a.reshape(NRANKS, T, P).transpose(0, 2, 1))

    return pack(eidx), pack(edst), pack(enrm), [int(x) for x in tile_win]


# ---------------------------------------------------------------- bass build
def _build_nc(cfg):
    NN, NB, B, DS, DC, DT, DL, H, O, T, tile_win, gsizes = (
        cfg["NN"],
        cfg["NB"],
        cfg["B"],
        cfg["DS"],
        cfg["DC"],
        cfg["DT"],
        cfg["DL"],
        cfg["H"],
        cfg["O"],
        cfg["T"],
        cfg["tile_win"],
        cfg["gsizes"],
    )
    NW = cdiv(NB, P)
    GB = B // NRANKS  # graphs per core
    NCOLS = NW * P

    nc = bacc.Bacc(
        get_trn_type() or "TRN2",
        target_bir_lowering=False,
        debug=False,
        num_devices=NRANKS,
    )
    dram = {}

    def inp(name, shape):
        dram[name] = nc.dram_tensor(name, list(shape), f32, kind="ExternalInput")
        return dram[name]

    t_x = inp("x", (NN, DS))
    t_eidx = nc.dram_tensor("eidx", [P, T], i32, kind="ExternalInput")
    dram["eidx"] = t_eidx
    t_edst = inp("edst", (P, T))
    t_enrm = inp("enrm", (P, T))
    t_iota = inp("iotaf", (P, P))
    t_ident = inp("ident", (P, P))
    t_chemT = inp("chemT", (DC, B))
    t_tgtT = inp("tgtT", (DT, B))
    t_cellT = inp("cellT", (DL, B))
    # params (all [rows, cols])
    for nm, shp in [
        ("W_conv1", (DS, H)),
        ("b_conv1", (H, 1)),
        ("g_bn1", (H, 1)),
        ("be_bn1", (H, 1)),
        ("W_conv2", (H, O)),
        ("b_conv2", (O, 1)),
        ("g_bn2", (O, 1)),
        ("be_bn2", (O, 1)),
        ("W_chem1", (DC, H)),
        ("b_chem1", (H, 1)),
        ("g_chem", (H, 1)),
        ("be_chem", (H, 1)),
        ("W_chem2", (H, O)),
        ("b_chem2", (O, 1)),
        ("W_tgt1", (DT, H)),
        ("b_tgt1", (H, 1)),
        ("g_tgt", (H, 1)),
        ("be_tgt", (H, 1)),
        ("W_tgt2", (H, O)),
        ("b_tgt2", (O, 1)),
        ("W_cell1", (DL, H)),
        ("b_cell1", (H, 1)),
        ("g_cell", (H, 1)),
        ("be_cell", (H, 1)),
        ("W_cell2", (H, O)),
        ("b_cell2", (O, 1)),
    ]:
        inp(nm, shp)

    o_drug = nc.dram_tensor("out_drug", [GB, O], f32, kind="ExternalOutput")
    o_chem = nc.dram_tensor("out_chem", [O, B], f32, kind="ExternalOutput")
    o_tgt = nc.dram_tensor("out_tgt", [O, B], f32, kind="ExternalOutput")
    o_cell = nc.dram_tensor("out_cell", [O, B], f32, kind="ExternalOutput")

    with tile.TileContext(nc) as tc, ExitStack() as ctx:
        cpool = ctx.enter_context(tc.tile_pool(name="cpool", bufs=1))
        idxp = ctx.enter_context(tc.tile_pool(name="idxp", bufs=1))
        msgp = ctx.enter_context(tc.tile_pool(name="msgp", bufs=8))
        sp = ctx.enter_context(tc.tile_pool(name="sp", bufs=4))
        bigp = ctx.enter_context(tc.tile_pool(name="bigp", bufs=2))
        xkp = ctx.enter_context(tc.tile_pool(name="xkp", bufs=2))
        wkp = ctx.enter_context(tc.tile_pool(name="wkp", bufs=16))
        brp = ctx.enter_context(tc.tile_pool(name="brp", bufs=1))
        smp = ctx.enter_context(tc.tile_pool(name="smp", bufs=4))
        aggps = ctx.enter_context(tc.tile_pool(name="aggps", bufs=2, space="PSUM"))
        trps = ctx.enter_context(tc.tile_pool(name="trps", bufs=1, space="PSUM"))
        brps = ctx.enter_context(tc.tile_pool(name="brps", bufs=4, space="PSUM"))
        dramp = ctx.enter_context(tc.tile_pool(name="dramp", bufs=1, space="DRAM"))

        # ---- constants / params to SBUF
        iota_f = cpool.tile([P, P], f32)
        nc.sync.dma_start(iota_f[:], t_iota[:])
        ident = cpool.tile([P, P], f32)
        nc.sync.dma_start(ident[:], t_ident[:])

        def load_col(name):
            t = cpool.tile([P, 1], f32, name=f"c_{name}")
            nc.sync.dma_start(t[: dram[name].shape[0]], dram[name][:])
            return t

        cols = {
            nm: load_col(nm)
            for nm in [
                "b_conv1",
                "g_bn1",
                "be_bn1",
                "b_conv2",
                "g_bn2",
                "be_bn2",
                "b_chem1",
                "g_chem",
                "be_chem",
                "b_chem2",
                "b_tgt1",
                "g_tgt",
                "be_tgt",
                "b_tgt2",
                "b_cell1",
                "g_cell",
                "be_cell",
                "b_cell2",
            ]
        }

        idx_t = idxp.tile([P, T], i32)
        nc.sync.dma_start(idx_t[:], t_eidx[:])
        dst_t = idxp.tile([P, T], f32)
        nc.sync.dma_start(dst_t[:], t_edst[:])
        nrm_t = idxp.tile([P, T], f32)
        nc.sync.dma_start(nrm_t[:], t_enrm[:])

        # ============ MLP branch helper (feature-major, full batch) =========
        def branch(xT, DIN, W1n, b1n, gn, ben, W2n, b2n, o_out, act1=AF.Tanh):
            K1 = DIN // P
            NBK = cdiv(B, 512)
            # load W1 k-slices
            w1s = []
            for k in range(K1):
                wt = wkp.tile([P, H], f32, tag="wk")
                nc.sync.dma_start(wt[:], dram[W1n][k * P : (k + 1) * P, :])
                w1s.append(wt)
            hT = brp.tile([P, B], f32, tag="brh", name=f"h_{W1n}")
            for nb_ in range(NBK):
                n0, n1 = nb_ * 512, min((nb_ + 1) * 512, B)
                pt = brps.tile([P, 512], f32, tag="pb512")
                for k in range(K1):
                    xk = xkp.tile([P, B], f32, tag="xk")
                    nc.sync.dma_start(xk[:], xT[k * P : (k + 1) * P, :])
                    nc.tensor.matmul(
                        pt[:, : n1 - n0],
                        w1s[k][:],
                        xk[:, n0:n1],
                        start=(k == 0),
                        stop=(k == K1 - 1),
                    )
                nc.scalar.activation(
                    hT[:, n0:n1], pt[:, : n1 - n0], act1, bias=cols[b1n][:], scale=1.0
                )
            # BN over full batch (local)
            nstat = cdiv(B, 512)
            stats = smp.tile([P, nstat * 6], f32, tag="stats")
            for j in range(nstat):
                c0, c1 = j * 512, min((j + 1) * 512, B)
                nc.vector.bn_stats(stats[:, j * 6 : (j + 1) * 6], hT[:, c0:c1])
            mv = smp.tile([P, 2], f32, tag="mv")
            nc.vector.bn_aggr(mv[:], stats[:, : nstat * 6])
            scale = smp.tile([P, 1], f32, tag="scl")
            shift = smp.tile([P, 1], f32, tag="shf")
            _bn_coeffs(nc, smp, mv, cols[gn], cols[ben], scale, shift)
            nc.vector.tensor_scalar(
                out=hT[:],
                in0=hT[:],
                scalar1=scale[:, :1],
                scalar2=shift[:, :1],
                op0=ALU.mult,
                op1=ALU.add,
            )
            # layer B: relu(W2.T @ hbn + b2)
            w2 = wkp.tile([P, O], f32, tag="wk")
            nc.sync.dma_start(w2[:], dram[W2n][:])
            for nb_ in range(NBK):
                n0, n1 = nb_ * 512, min((nb_ + 1) * 512, B)
                pt = brps.tile([P, 512], f32, tag="pb512")
                nc.tensor.matmul(
                    pt[:, : n1 - n0], w2[:], hT[:, n0:n1], start=True, stop=True
                )
                ot = sp.tile([P, 512], f32, tag="brout")
                nc.scalar.activation(
                    ot[:, : n1 - n0], pt[:, : n1 - n0], AF.Relu, bias=cols[b2n][:], scale=1.0
                )
                nc.sync.dma_start(o_out[:, n0:n1], ot[:, : n1 - n0])

        def _bn_coeffs(nc, pool, mv, g_ap, be_ap, scale, shift):
            # scale = g / sqrt(var+eps); shift = be - mean*scale
            tmp = pool.tile([P, 1], f32, tag="tmp1")
            nc.vector.tensor_scalar_add(tmp[:], mv[:, 1:2], EPS)
            sq = pool.tile([P, 1], f32, tag="tmp2")
            nc.scalar.activation(sq[:], tmp[:], AF.Sqrt)
            rc = pool.tile([P, 1], f32, tag="tmp3")
            nc.vector.reciprocal(rc[:], sq[:])
            nc.vector.tensor_tensor(out=scale[:], in0=rc[:], in1=g_ap[:, :1], op=ALU.mult)
            nc.vector.tensor_tensor(out=tmp[:], in0=mv[:, 0:1], in1=scale[:], op=ALU.mult)
            nc.vector.tensor_tensor(out=shift[:], in0=be_ap[:, :1], in1=tmp[:], op=ALU.subtract)

        branch(t_chemT, DC, "W_chem1", "b_chem1", "g_chem", "be_chem", "W_chem2", "b_chem2", o_chem)
        branch(t_tgtT, DT, "W_tgt1", "b_tgt1", "g_tgt", "be_tgt", "W_tgt2", "b_tgt2", o_tgt)
        branch(t_cellT, DL, "W_cell1", "b_cell1", "g_cell", "be_cell", "W_cell2", "b_cell2", o_cell)

        # ================== GCN aggregation (one layer) =====================
        def aggregate(table_ap, F_in, haggT):
            """haggT [F_in, NCOLS] feat-major aggregation of norm-weighted
            neighbor features, per dst window."""
            pt = None
            for t in range(T):
                wi = tile_win[t]
                first = t == 0 or tile_win[t - 1] != wi
                last = t == T - 1 or tile_win[t + 1] != wi
                msg = msgp.tile([P, F_in], f32, tag="msg")
                nc.gpsimd.indirect_dma_start(
                    out=msg[:],
                    out_offset=None,
                    in_=table_ap,
                    in_offset=bass.IndirectOffsetOnAxis(ap=idx_t[:, t : t + 1], axis=0),
                )
                s_tile = sp.tile([P, P], f32, tag="S")
                nc.vector.tensor_scalar(
                    out=s_tile[:],
                    in0=iota_f[:],
                    scalar1=dst_t[:, t : t + 1],
                    scalar2=nrm_t[:, t : t + 1],
                    op0=ALU.is_equal,
                    op1=ALU.mult,
                )
                if first:
                    pt = aggps.tile([P, P], f32, tag="aggps")
                nc.tensor.matmul(
                    pt[:F_in, :], msg[:], s_tile[:], start=first, stop=last
                )
                if last:
                    nc.scalar.activation(
                        haggT[:F_in, wi * P : (wi + 1) * P], pt[:F_in, :], AF.Copy
                    )

        def wmm_relu(haggT, F_in, Wn, bn_, outT, F_out):
            # outT[F_out, NCOLS] = relu(W.T @ haggT + b)
            wt = wkp.tile([P, F_out], f32, tag="wk")
            nc.sync.dma_start(wt[:F_in, :], dram[Wn][:])
            for j in range(cdiv(NCOLS, 512)):
                c0, c1 = j * 512, min((j + 1) * 512, NCOLS)
                pt = brps.tile([P, 512], f32, tag="pb512")
                nc.tensor.matmul(
                    pt[:F_out, : c1 - c0], wt[:F_in, :], haggT[:F_in, c0:c1],
                    start=True, stop=True,
                )
                nc.scalar.activation(
                    outT[:F_out, c0:c1], pt[:F_out, : c1 - c0], AF.Relu,
                    bias=cols[bn_][:], scale=1.0,
                )

        def bn_global(hT, F_out, gn, ben):
            # batch-norm over ALL nodes (cross-core AllReduce of stats)
            nstat = cdiv(NB, 512)
            stats = smp.tile([P, nstat * 6], f32, tag="stats")
            for j in range(nstat):
                c0, c1 = j * 512, min((j + 1) * 512, NB)
                nc.vector.bn_stats(stats[:, j * 6 : (j + 1) * 6], hT[:, c0:c1])
            mv = smp.tile([P, 2], f32, tag="mv")
            nc.vector.bn_aggr(mv[:], stats[:, : nstat * 6])
            # ar_in = [mean/8, (var+mean^2)/8]
            ar_in = smp.tile([P, 2], f32, tag="arin")
            msq = smp.tile([P, 1], f32, tag="tmp1")
            nc.vector.tensor_tensor(out=msq[:], in0=mv[:, 0:1], in1=mv[:, 0:1], op=ALU.mult)
            nc.vector.tensor_tensor(out=ar_in[:, 1:2], in0=mv[:, 1:2], in1=msq[:], op=ALU.add)
            nc.vector.tensor_copy(ar_in[:, 0:1], mv[:, 0:1])
            nc.vector.tensor_scalar_mul(ar_in[:], ar_in[:], 1.0 / NRANKS)
            ar_i = dramp.tile([P, 2], f32, tag="ari", name=f"ari_{gn}")
            nc.gpsimd.dma_start(ar_i[:], ar_in[:])
            ar_o = dramp.tile([P, 2], f32, tag="aro", name=f"aro_{gn}")
            nc.gpsimd.collective_compute(
                "AllReduce",
                ALU.add,
                replica_groups=[list(range(NRANKS))],
                ins=[ar_i.opt()],
                outs=[ar_o.opt()],
            )
            gstat = smp.tile([P, 2], f32, tag="gstat")
            nc.sync.dma_start(gstat[:], ar_o[:])
            # var = E[x^2] - mu^2
            mv2 = smp.tile([P, 2], f32, tag="mv2")
            nc.vector.tensor_tensor(out=msq[:], in0=gstat[:, 0:1], in1=gstat[:, 0:1], op=ALU.mult)
            nc.vector.tensor_tensor(out=mv2[:, 1:2], in0=gstat[:, 1:2], in1=msq[:], op=ALU.subtract)
            nc.vector.tensor_copy(mv2[:, 0:1], gstat[:, 0:1])
            scale = smp.tile([P, 1], f32, tag="scl")
            shift = smp.tile([P, 1], f32, tag="shf")
            _bn_coeffs(nc, smp, mv2, cols[gn], cols[ben], scale, shift)
            nc.vector.tensor_scalar(
                out=hT[:F_out, :NCOLS],
                in0=hT[:F_out, :NCOLS],
                scalar1=scale[:, :1],
                scalar2=shift[:, :1],
                op0=ALU.mult,
                op1=ALU.add,
            )

        # ---------------- layer 1 ----------------
        hagg1 = bigp.tile([P, NCOLS], f32, tag="big", name="hagg1")
        aggregate(t_x[:], DS, hagg1)
        h1rT = bigp.tile([P, NCOLS], f32, tag="big", name="h1rT")
        wmm_relu(hagg1, DS, "W_conv1", "b_conv1", h1rT, H)
        bn_global(h1rT, H, "g_bn1", "be_bn1")

        # transpose h1rT -> node-major shard, AllGather into full table
        ag_in = dramp.tile([NB, H], f32, tag="agin")
        for w in range(NW):
            pt = trps.tile([P, P], f32, tag="trp")
            nc.tensor.transpose(pt[:], h1rT[:, w * P : (w + 1) * P], ident[:])
            st = sp.tile([P, P], f32, tag="trs")
            nc.scalar.activation(st[:], pt[:], AF.Copy)
            r0 = w * P
            r1 = min(NB, r0 + P)
            nc.sync.dma_start(ag_in[r0:r1, :], st[: r1 - r0, :])
        h1_full = dramp.tile([NB * NRANKS, H], f32, tag="h1full")
        nc.gpsimd.collective_compute(
            "AllGather",
            ALU.bypass,
            replica_groups=[list(range(NRANKS))],
            ins=[ag_in.opt()],
            outs=[h1_full.opt()],
        )

        # ---------------- layer 2 ----------------
        hagg2 = bigp.tile([P, NCOLS], f32, tag="big", name="hagg2")
        aggregate(h1_full[:], H, hagg2)
        h2rT = bigp.tile([P, NCOLS], f32, tag="big", name="h2rT")
        wmm_relu(hagg2, H, "W_conv2", "b_conv2", h2rT, O)
        bn_global(h2rT, O, "g_bn2", "be_bn2")

        # ---------------- segment-max pooling ----------------
        pooled = sp.tile([P, max(P, GB)], f32, tag="pooled")
        nc.vector.memset(pooled[:], 0.0)
        s0 = 0
        for g in range(GB):
            e0 = s0 + gsizes[g]
            nc.vector.reduce_max(
                pooled[:, g : g + 1], h2rT[:, s0:e0], axis=mybir.AxisListType.X
            )
            s0 = e0
        for j in range(cdiv(GB, P)):
            c0, c1 = j * P, min((j + 1) * P, GB)
            pt = trps.tile([P, P], f32, tag="trp")
            nc.tensor.transpose(pt[:], pooled[:, c0 : c0 + P], ident[:])
            st = sp.tile([P, P], f32, tag="trs")
            nc.scalar.activation(st[:], pt[:], AF.Copy)
            nc.sync.dma_start(o_drug[c0:c1, :], st[: c1 - c0, :])

    nc.compile()
    return nc


_NC_CACHE = {}


def _get_nc(key, cfg):
    if key not in _NC_CACHE:
        _NC_CACHE[key] = _build_nc(cfg)
    return _NC_CACHE[key]


# ---------------------------------------------------------------- entry point
def kernel(
    drug_stru_feature,
    drug_adj,
    ibatch,
    drug_chem_feature,
    drug_target_feature,
    gexpr_data,
    **params,
):
    x = np.ascontiguousarray(np.asarray(drug_stru_feature, np.float32))
    adj = np.asarray(drug_adj)
    ib = np.asarray(ibatch)
    NN, DS = x.shape
    B = drug_chem_feature.shape[0]
    DC = drug_chem_feature.shape[1]
    DT = drug_target_feature.shape[1]
    DL = gexpr_data.shape[1]
    H = params["W_conv1"].shape[1]
    O = params["W_conv2"].shape[1]
    NB = NN // NRANKS
    GB = B // NRANKS

    # --- graph preprocessing (host): self loops, degrees, symmetric norm
    src = np.asarray(adj[0], np.int64)
    dst = np.asarray(adj[1], np.int64)
    deg = np.bincount(dst, minlength=NN).astype(np.float32) + 1.0
    dinv = 1.0 / np.sqrt(deg)
    src_g = np.concatenate([src, np.arange(NN, dtype=np.int64)])
    dst_g = np.concatenate([dst, np.arange(NN, dtype=np.int64)])
    norm_g = (dinv[src_g] * dinv[dst_g]).astype(np.float32)

    eidx, edst, enrm, tile_win = _build_plan(src_g, dst_g, norm_g, NB)
    T = eidx.shape[2]

    # --- pooling schedule: per-core graph sizes (must be uniform across cores)
    counts = np.bincount(ib, minlength=B).astype(np.int64)
    csz = counts.reshape(NRANKS, GB)
    assert (csz == csz[0]).all(), "graph-size pattern must repeat per core"
    assert counts.reshape(NRANKS, -1).sum(axis=1)[0] == NB
    gsizes = [int(v) for v in csz[0]]

    cfg = dict(
        NN=NN, NB=NB, B=B, DS=DS, DC=DC, DT=DT, DL=DL, H=H, O=O,
        T=T, tile_win=tile_win, gsizes=gsizes,
    )
    key = (NN, NB, B, DS, DC, DT, DL, H, O, T, tuple(tile_win), tuple(gsizes))
    nc = _get_nc(key, cfg)

    iota_f = np.tile(np.arange(P, dtype=np.float32)[None, :], (P, 1))
    ident = np.eye(P, dtype=np.float32)
    chemT = np.ascontiguousarray(np.asarray(drug_chem_feature, np.float32).T)
    tgtT = np.ascontiguousarray(np.asarray(drug_target_feature, np.float32).T)
    cellT = np.ascontiguousarray(np.asarray(gexpr_data, np.float32).T)

    common = dict(
        x=x, iotaf=iota_f, ident=ident, chemT=chemT, tgtT=tgtT, cellT=cellT
    )
    for k, v in params.items():
        v = np.asarray(v, np.float32)
        if v.ndim == 1:
            v = v[:, None]
        common[k] = np.ascontiguousarray(v)

    in_maps = []
    for c in range(NRANKS):
        m = dict(common)
        m["eidx"] = np.ascontiguousarray(eidx[c])
        m["edst"] = np.ascontiguousarray(edst[c])
        m["enrm"] = np.ascontiguousarray(enrm[c])
        in_maps.append(m)

    res = run_bass_kernel_spmd(nc, in_maps, core_ids=list(range(NRANKS)))
    outs = res.results

    x_drug = np.concatenate([outs[c]["out_drug"] for c in range(NRANKS)], axis=0)
    x_chem = np.ascontiguousarray(outs[0]["out_chem"].T)
    x_tgt = np.ascontiguousarray(outs[0]["out_tgt"].T)
    x_cell = np.ascontiguousarray(outs[0]["out_cell"].T)
    return (x_drug, x_chem, x_tgt, x_cell)


# revision 4
# speedup vs baseline: 139933.5861x; 139933.5861x over previous
"""Trainium2 Bass kernel for nn_BioEncoder (GCN + 3 MLP branches), 8 cores.

Sharding: nodes/edges by dst block across 8 cores (graph-parallel); the small
MLP branches are replicated (full batch) on every core; weights replicated.

GCN aggregation: edges sorted by dst window (128), per-tile indirect-DMA row
gather (128 rows/call, one per partition) + iota/tensor_scalar selection
matrix + PE matmul scatter-add into PSUM (feature-major), giving
out^T = sum_e norm_e * h[src_e] per dst window.  Symmetric normalization is
folded into the per-edge S-matrix weights (norm = dinv[src]*dinv[dst]).
BatchNorm batch stats via bn_stats/bn_aggr + AllReduce; h1 table AllGathered
for layer-2 gathers.
"""

import numpy as np

import concourse.bacc as bacc
import concourse.bass as bass
import concourse.mybir as mybir
import concourse.tile as tile
from contextlib import ExitStack
from concourse._compat import cdiv, get_trn_type
from concourse.bass_utils import run_bass_kernel_spmd

P = 128
NRANKS = 8
f32 = mybir.dt.float32
i32 = mybir.dt.int32
AF = mybir.ActivationFunctionType
ALU = mybir.AluOpType
EPS = 1e-5


# ---------------------------------------------------------------- host prep
def _build_plan(src_g, dst_g, norm_g, nb):
    """Global (self-loop-augmented) edges -> per-core packed tile streams with
    a schedule (tile->window map) UNIFORM across cores (SPMD: one program).

    Returns (eidx [8,128,T], edst [8,128,T], enrm [8,128,T], tile_win [T])."""
    nw = cdiv(nb, P)
    core = dst_g // nb
    dloc = dst_g - core * nb
    win = dloc // P
    # per (core, window) edge lists
    counts = np.zeros((NRANKS, nw), np.int64)
    np.add.at(counts, (core, win), 1)
    tiles_w = np.maximum(1, -(-counts.max(axis=0) // P))  # global per-window tiles
    T = int(tiles_w.sum())

    eidx = np.zeros((NRANKS, T * P), np.int32)
    edst = -np.ones((NRANKS, T * P), np.float32)
    enrm = np.zeros((NRANKS, T * P), np.float32)
    tile_win = np.repeat(np.arange(nw), tiles_w)

    # window start offset (in padded slots) per window
    wstart = np.concatenate([[0], np.cumsum(tiles_w)])[:-1] * P

    order = np.lexsort((win, core))
    s_s, d_s, n_s, c_s, w_s = (
        src_g[order],
        (dloc - win * P)[order],
        norm_g[order],
        core[order],
        win[order],
    )
    # position within (core, window) group
    grp = c_s * nw + w_s
    first = np.ones(len(grp), bool)
    first[1:] = grp[1:] != grp[:-1]
    gstart = np.where(first)[0]
    gid = np.cumsum(first) - 1
    pos_in_grp = np.arange(len(grp)) - gstart[gid]
    slot = wstart[w_s] + pos_in_grp
    eidx[c_s, slot] = s_s
    edst[c_s, slot] = d_s
    enrm[c_s, slot] = n_s

    def pack(a):
        # slot i -> tile i//128, partition i%128 ; SBUF layout [128, T]
        return np.ascontiguousarray(a.reshape(NRANKS, T, P).transpose(0, 2, 1))

    return pack(eidx), pack(edst), pack(enrm), [int(x) for x in tile_win]


# ---------------------------------------------------------------- bass build
def _build_nc(cfg):
    NN, NB, B, DS, DC, DT, DL, H, O, T, tile_win, gsizes = (
        cfg["NN"],
        cfg["NB"],
        cfg["B"],
        cfg["DS"],
        cfg["DC"],
        cfg["DT"],
        cfg["DL"],
        cfg["H"],
        cfg["O"],
        cfg["T"],
        cfg["tile_win"],
        cfg["gsizes"],
    )
    NW = cdiv(NB, P)
    GB = B // NRANKS  # graphs per core
    NCOLS = NW * P

    nc = bacc.Bacc(
        get_trn_type() or "TRN2",
        target_bir_lowering=False,
        debug=False,
        num_devices=NRANKS,
    )
    dram = {}

    def inp(name, shape):
        dram[name] = nc.dram_tensor(name, list(shape), f32, kind="ExternalInput")
        return dram[name]

    t_x = inp("x", (NN, DS))
    t_eidx = nc.dram_tensor("eidx", [P, T], i32, kind="ExternalInput")
    dram["eidx"] = t_eidx
    t_edst = inp("edst", (P, T))
    t_enrm = inp("enrm", (P, T))
    t_iota = inp("iotaf", (P, P))
    t_ident = inp("ident", (P, P))
    t_chemT = inp("chemT", (DC, B))
    t_tgtT = inp("tgtT", (DT, B))
    t_cellT = inp("cellT", (DL, B))
    # params (all [rows, cols])
    for nm, shp in [
        ("W_conv1", (DS, H)),
        ("b_conv1", (H, 1)),
        ("g_bn1", (H, 1)),
        ("be_bn1", (H, 1)),
        ("W_conv2", (H, O)),
        ("b_conv2", (O, 1)),
        ("g_bn2", (O, 1)),
        ("be_bn2", (O, 1)),
        ("W_chem1", (DC, H)),
        ("b_chem1", (H, 1)),
        ("g_chem", (H, 1)),
        ("be_chem", (H, 1)),
        ("W_chem2", (H, O)),
        ("b_chem2", (O, 1)),
        ("W_tgt1", (DT, H)),
        ("b_tgt1", (H, 1)),
        ("g_tgt", (H, 1)),
        ("be_tgt", (H, 1)),
        ("W_tgt2", (H, O)),
        ("b_tgt2", (O, 1)),
        ("W_cell1", (DL, H)),
        ("b_cell1", (H, 1)),
        ("g_cell", (H, 1)),
        ("be_cell", (H, 1)),
        ("W_cell2", (H, O)),
        ("b_cell2", (O, 1)),
    ]:
        inp(nm, shp)

    o_drug = nc.dram_tensor("out_drug", [GB, O], f32, kind="ExternalOutput")
    o_chem = nc.dram_tensor("out_chem", [O, B], f32, kind="ExternalOutput")
    o_tgt = nc.dram_tensor("out_tgt", [O, B], f32, kind="ExternalOutput")
    o_cell = nc.dram_tensor("out_cell", [O, B], f32, kind="ExternalOutput")

    with tile.TileContext(nc) as tc, ExitStack() as ctx:
        cpool = ctx.enter_context(tc.tile_pool(name="cpool", bufs=1))
        idxp = ctx.enter_context(tc.tile_pool(name="idxp", bufs=1))
        msgp = ctx.enter_context(tc.tile_pool(name="msgp", bufs=8))
        sp = ctx.enter_context(tc.tile_pool(name="sp", bufs=4))
        bigp = ctx.enter_context(tc.tile_pool(name="bigp", bufs=2))
        xkp = ctx.enter_context(tc.tile_pool(name="xkp", bufs=2))
        wkp = ctx.enter_context(tc.tile_pool(name="wkp", bufs=16))
        brp = ctx.enter_context(tc.tile_pool(name="brp", bufs=1))
        smp = ctx.enter_context(tc.tile_pool(name="smp", bufs=4))
        aggps = ctx.enter_context(tc.tile_pool(name="aggps", bufs=2, space="PSUM"))
        trps = ctx.enter_context(tc.tile_pool(name="trps", bufs=1, space="PSUM"))
        brps = ctx.enter_context(tc.tile_pool(name="brps", bufs=4, space="PSUM"))
        dramp = ctx.enter_context(tc.tile_pool(name="dramp", bufs=1, space="DRAM"))

        # ---- constants / params to SBUF
        iota_f = cpool.tile([P, P], f32)
        nc.sync.dma_start(iota_f[:], t_iota[:])
        ident = cpool.tile([P, P], f32)
        nc.sync.dma_start(ident[:], t_ident[:])

        def load_col(name):
            t = cpool.tile([P, 1], f32, name=f"c_{name}")
            nc.sync.dma_start(t[: dram[name].shape[0]], dram[name][:])
            return t

        cols = {
            nm: load_col(nm)
            for nm in [
                "b_conv1",
                "g_bn1",
                "be_bn1",
                "b_conv2",
                "g_bn2",
                "be_bn2",
                "b_chem1",
                "g_chem",
                "be_chem",
                "b_chem2",
                "b_tgt1",
                "g_tgt",
                "be_tgt",
                "b_tgt2",
                "b_cell1",
                "g_cell",
                "be_cell",
                "b_cell2",
            ]
        }

        idx_t = idxp.tile([P, T], i32)
        nc.sync.dma_start(idx_t[:], t_eidx[:])
        dst_t = idxp.tile([P, T], f32)
        nc.sync.dma_start(dst_t[:], t_edst[:])
        nrm_t = idxp.tile([P, T], f32)
        nc.sync.dma_start(nrm_t[:], t_enrm[:])

        # ============ MLP branch helper (feature-major, full batch) =========
        def branch(xT, DIN, W1n, b1n, gn, ben, W2n, b2n, o_out, act1=AF.Tanh):
            K1 = DIN // P
            NBK = cdiv(B, 512)
            # load W1 k-slices
            w1s = []
            for k in range(K1):
                wt = wkp.tile([P, H], f32, tag="wk")
                nc.sync.dma_start(wt[:], dram[W1n][k * P : (k + 1) * P, :])
                w1s.append(wt)
            hT = brp.tile([P, B], f32, tag="brh", name=f"h_{W1n}")
            for nb_ in range(NBK):
                n0, n1 = nb_ * 512, min((nb_ + 1) * 512, B)
                pt = brps.tile([P, 512], f32, tag="pb512")
                for k in range(K1):
                    xk = xkp.tile([P, B], f32, tag="xk")
                    nc.sync.dma_start(xk[:], xT[k * P : (k + 1) * P, :])
                    nc.tensor.matmul(
                        pt[:, : n1 - n0],
                        w1s[k][:],
                        xk[:, n0:n1],
                        start=(k == 0),
                        stop=(k == K1 - 1),
                    )
                nc.scalar.activation(
                    hT[:, n0:n1], pt[:, : n1 - n0], act1, bias=cols[b1n][:], scale=1.0
                )
            # BN over full batch (local)
            nstat = cdiv(B, 512)
            stats = smp.tile([P, nstat * 6], f32, tag="stats")
            for j in range(nstat):
                c0, c1 = j * 512, min((j + 1) * 512, B)
                nc.vector.bn_stats(stats[:, j * 6 : (j + 1) * 6], hT[:, c0:c1])
            mv = smp.tile([P, 2], f32, tag="mv")
            nc.vector.bn_aggr(mv[:], stats[:, : nstat * 6])
            scale = smp.tile([P, 1], f32, tag="scl")
            shift = smp.tile([P, 1], f32, tag="shf")
            _bn_coeffs(nc, smp, mv, cols[gn], cols[ben], scale, shift)
            nc.vector.tensor_scalar(
                out=hT[:],
                in0=hT[:],
                scalar1=scale[:, :1],
                scalar2=shift[:, :1],
                op0=ALU.mult,
                op1=ALU.add,
            )
            # layer B: relu(W2.T @ hbn + b2)
            w2 = wkp.tile([P, O], f32, tag="wk")
            nc.sync.dma_start(w2[:], dram[W2n][:])
            for nb_ in range(NBK):
                n0, n1 = nb_ * 512, min((nb_ + 1) * 512, B)
                pt = brps.tile([P, 512], f32, tag="pb512")
                nc.tensor.matmul(
                    pt[:, : n1 - n0], w2[:], hT[:, n0:n1], start=True, stop=True
                )
                ot = sp.tile([P, 512], f32, tag="brout")
                nc.scalar.activation(
                    ot[:, : n1 - n0], pt[:, : n1 - n0], AF.Relu, bias=cols[b2n][:], scale=1.0
                )
                nc.sync.dma_start(o_out[:, n0:n1], ot[:, : n1 - n0])

        def _bn_coeffs(nc, pool, mv, g_ap, be_ap, scale, shift):
            # scale = g / sqrt(var+eps); shift = be - mean*scale
            tmp = pool.tile([P, 1], f32, tag="tmp1")
            nc.vector.tensor_scalar_add(tmp[:], mv[:, 1:2], EPS)
            sq = pool.tile([P, 1], f32, tag="tmp2")
            nc.scalar.activation(sq[:], tmp[:], AF.Sqrt)
            rc = pool.tile([P, 1], f32, tag="tmp3")
            nc.vector.reciprocal(rc[:], sq[:])
            nc.vector.tensor_tensor(out=scale[:], in0=rc[:], in1=g_ap[:, :1], op=ALU.mult)
            nc.vector.tensor_tensor(out=tmp[:], in0=mv[:, 0:1], in1=scale[:], op=ALU.mult)
            nc.vector.tensor_tensor(out=shift[:], in0=be_ap[:, :1], in1=tmp[:], op=ALU.subtract)

        branch(t_chemT, DC, "W_chem1", "b_chem1", "g_chem", "be_chem", "W_chem2", "b_chem2", o_chem)
        branch(t_tgtT, DT, "W_tgt1", "b_tgt1", "g_tgt", "be_tgt", "W_tgt2", "b_tgt2", o_tgt)
        branch(t_cellT, DL, "W_cell1", "b_cell1", "g_cell", "be_cell", "W_cell2", "b_cell2", o_cell)

        # ================== GCN aggregation (one layer) =====================
        def aggregate(table_ap, F_in, haggT):
            """haggT [F_in, NCOLS] feat-major aggregation of norm-weighted
            neighbor features, per dst window."""
            pt = None
            for t in range(T):
                wi = tile_win[t]
                first = t == 0 or tile_win[t - 1] != wi
                last = t == T - 1 or tile_win[t + 1] != wi
                msg = msgp.tile([P, F_in], f32, tag="msg")
                nc.gpsimd.indirect_dma_start(
                    out=msg[:],
                    out_offset=None,
                    in_=table_ap,
                    in_offset=bass.IndirectOffsetOnAxis(ap=idx_t[:, t : t + 1], axis=0),
                )
                s_tile = sp.tile([P, P], f32, tag="S")
                nc.vector.tensor_scalar(
                    out=s_tile[:],
                    in0=iota_f[:],
                    scalar1=dst_t[:, t : t + 1],
                    scalar2=nrm_t[:, t : t + 1],
                    op0=ALU.is_equal,
                    op1=ALU.mult,
                )
                if first:
                    pt = aggps.tile([P, P], f32, tag="aggps")
                nc.tensor.matmul(
                    pt[:F_in, :], msg[:], s_tile[:], start=first, stop=last
                )
                if last:
                    nc.scalar.activation(
                        haggT[:F_in, wi * P : (wi + 1) * P], pt[:F_in, :], AF.Copy
                    )

        def wmm_relu(haggT, F_in, Wn, bn_, outT, F_out):
            # outT[F_out, NCOLS] = relu(W.T @ haggT + b)
            wt = wkp.tile([P, F_out], f32, tag="wk")
            nc.sync.dma_start(wt[:F_in, :], dram[Wn][:])
            for j in range(cdiv(NCOLS, 512)):
                c0, c1 = j * 512, min((j + 1) * 512, NCOLS)
                pt = brps.tile([P, 512], f32, tag="pb512")
                nc.tensor.matmul(
                    pt[:F_out, : c1 - c0], wt[:F_in, :], haggT[:F_in, c0:c1],
                    start=True, stop=True,
                )
                nc.scalar.activation(
                    outT[:F_out, c0:c1], pt[:F_out, : c1 - c0], AF.Relu,
                    bias=cols[bn_][:], scale=1.0,
                )

        def bn_global(hT, F_out, gn, ben):
            # batch-norm over ALL nodes (cross-core AllReduce of stats)
            nstat = cdiv(NB, 512)
            stats = smp.tile([P, nstat * 6], f32, tag="stats")
            for j in range(nstat):
                c0, c1 = j * 512, min((j + 1) * 512, NB)
                nc.vector.bn_stats(stats[:, j * 6 : (j + 1) * 6], hT[:, c0:c1])
            mv = smp.tile([P, 2], f32, tag="mv")
            nc.vector.bn_aggr(mv[:], stats[:, : nstat * 6])
            # ar_in = [mean/8, (var+mean^2)/8]
            ar_in = smp.tile([P, 2], f32, tag="arin")
            msq = smp.tile([P, 1], f32, tag="tmp1")
            nc.vector.tensor_tensor(out=msq[:], in0=mv[:, 0:1], in1=mv[:, 0:1], op=ALU.mult)
            nc.vector.tensor_tensor(out=ar_in[:, 1:2], in0=mv[:, 1:2], in1=msq[:], op=ALU.add)
            nc.vector.tensor_copy(ar_in[:, 0:1], mv[:, 0:1])
            nc.vector.tensor_scalar_mul(ar_in[:], ar_in[:], 1.0 / NRANKS)
            ar_i = dramp.tile([P, 2], f32, tag="ari", name=f"ari_{gn}")
            nc.gpsimd.dma_start(ar_i[:], ar_in[:])
            ar_o = dramp.tile([P, 2], f32, tag="aro", name=f"aro_{gn}")
            nc.gpsimd.collective_compute(
                "AllReduce",
                ALU.add,
                replica_groups=[list(range(NRANKS))],
                ins=[ar_i.opt()],
                outs=[ar_o.opt()],
            )
            gstat = smp.tile([P, 2], f32, tag="gstat")
            nc.sync.dma_start(gstat[:], ar_o[:])
            # var = E[x^2] - mu^2
            mv2 = smp.tile([P, 2], f32, tag="mv2")
            nc.vector.tensor_tensor(out=msq[:], in0=gstat[:, 0:1], in1=gstat[:, 0:1], op=ALU.mult)
            nc.vector.tensor_tensor(out=mv2[:, 1:2], in0=gstat[:, 1:2], in1=msq[:], op=ALU.subtract)
            nc.vector.tensor_copy(mv2[:, 0:1], gstat[:, 0:1])
            scale = smp.tile([P, 1], f32, tag="scl")
            shift = smp.tile([P, 1], f32, tag="shf")
            _bn_coeffs(nc, smp, mv2, cols[gn], cols[ben], scale, shift)
            nc.vector.tensor_scalar(
                out=hT[:F_out, :NCOLS],
                in0=hT[:F_out, :NCOLS],
                scalar1=scale[:, :1],
                scalar2=shift[:, :1],
                op0=ALU.mult,
                op1=ALU.add,
            )

        # ---------------- layer 1 ----------------
        hagg1 = bigp.tile([P, NCOLS], f32, tag="big", name="hagg1")
        aggregate(t_x[:], DS, hagg1)
        h1rT = bigp.tile([P, NCOLS], f32, tag="big", name="h1rT")
        wmm_relu(hagg1, DS, "W_conv1", "b_conv1", h1rT, H)
        bn_global(h1rT, H, "g_bn1", "be_bn1")

        # transpose h1rT -> node-major shard, AllGather into full table
        ag_in = dramp.tile([NB, H], f32, tag="agin")
        for w in range(NW):
            pt = trps.tile([P, P], f32, tag="trp")
            nc.tensor.transpose(pt[:], h1rT[:, w * P : (w + 1) * P], ident[:])
            st = sp.tile([P, P], f32, tag="trs")
            nc.scalar.activation(st[:], pt[:], AF.Copy)
            r0 = w * P
            r1 = min(NB, r0 + P)
            nc.sync.dma_start(ag_in[r0:r1, :], st[: r1 - r0, :])
        h1_full = dramp.tile([NB * NRANKS, H], f32, tag="h1full")
        nc.gpsimd.collective_compute(
            "AllGather",
            ALU.bypass,
            replica_groups=[list(range(NRANKS))],
            ins=[ag_in.opt()],
            outs=[h1_full.opt()],
        )

        # ---------------- layer 2 ----------------
        hagg2 = bigp.tile([P, NCOLS], f32, tag="big", name="hagg2")
        aggregate(h1_full[:], H, hagg2)
        h2rT = bigp.tile([P, NCOLS], f32, tag="big", name="h2rT")
        wmm_relu(hagg2, H, "W_conv2", "b_conv2", h2rT, O)
        bn_global(h2rT, O, "g_bn2", "be_bn2")

        # ---------------- segment-max pooling ----------------
        pooled = sp.tile([P, max(P, GB)], f32, tag="pooled")
        nc.vector.memset(pooled[:], 0.0)
        s0 = 0
        for g in range(GB):
            e0 = s0 + gsizes[g]
            nc.vector.reduce_max(
                pooled[:, g : g + 1], h2rT[:, s0:e0], axis=mybir.AxisListType.X
            )
            s0 = e0
        for j in range(cdiv(GB, P)):
            c0, c1 = j * P, min((j + 1) * P, GB)
            pt = trps.tile([P, P], f32, tag="trp")
            nc.tensor.transpose(pt[:], pooled[:, c0 : c0 + P], ident[:])
            st = sp.tile([P, P], f32, tag="trs")
            nc.scalar.activation(st[:], pt[:], AF.Copy)
            nc.sync.dma_start(o_drug[c0:c1, :], st[: c1 - c0, :])

    nc.compile()
    return nc


_NC_CACHE = {}
_LAST_IN_MAPS = None


def _get_nc(key, cfg):
    if key not in _NC_CACHE:
        _NC_CACHE[key] = _build_nc(cfg)
    return _NC_CACHE[key]


# ---------------------------------------------------------------- entry point
def kernel(
    drug_stru_feature,
    drug_adj,
    ibatch,
    drug_chem_feature,
    drug_target_feature,
    gexpr_data,
    **params,
):
    x = np.ascontiguousarray(np.asarray(drug_stru_feature, np.float32))
    adj = np.asarray(drug_adj)
    ib = np.asarray(ibatch)
    NN, DS = x.shape
    B = drug_chem_feature.shape[0]
    DC = drug_chem_feature.shape[1]
    DT = drug_target_feature.shape[1]
    DL = gexpr_data.shape[1]
    H = params["W_conv1"].shape[1]
    O = params["W_conv2"].shape[1]
    NB = NN // NRANKS
    GB = B // NRANKS

    # --- graph preprocessing (host): self loops, degrees, symmetric norm
    src = np.asarray(adj[0], np.int64)
    dst = np.asarray(adj[1], np.int64)
    deg = np.bincount(dst, minlength=NN).astype(np.float32) + 1.0
    dinv = 1.0 / np.sqrt(deg)
    src_g = np.concatenate([src, np.arange(NN, dtype=np.int64)])
    dst_g = np.concatenate([dst, np.arange(NN, dtype=np.int64)])
    norm_g = (dinv[src_g] * dinv[dst_g]).astype(np.float32)

    eidx, edst, enrm, tile_win = _build_plan(src_g, dst_g, norm_g, NB)
    T = eidx.shape[2]

    # --- pooling schedule: per-core graph sizes (must be uniform across cores)
    counts = np.bincount(ib, minlength=B).astype(np.int64)
    csz = counts.reshape(NRANKS, GB)
    assert (csz == csz[0]).all(), "graph-size pattern must repeat per core"
    assert counts.reshape(NRANKS, -1).sum(axis=1)[0] == NB
    gsizes = [int(v) for v in csz[0]]

    cfg = dict(
        NN=NN, NB=NB, B=B, DS=DS, DC=DC, DT=DT, DL=DL, H=H, O=O,
        T=T, tile_win=tile_win, gsizes=gsizes,
    )
    key = (NN, NB, B, DS, DC, DT, DL, H, O, T, tuple(tile_win), tuple(gsizes))
    nc = _get_nc(key, cfg)

    iota_f = np.tile(np.arange(P, dtype=np.float32)[None, :], (P, 1))
    ident = np.eye(P, dtype=np.float32)
    chemT = np.ascontiguousarray(np.asarray(drug_chem_feature, np.float32).T)
    tgtT = np.ascontiguousarray(np.asarray(drug_target_feature, np.float32).T)
    cellT = np.ascontiguousarray(np.asarray(gexpr_data, np.float32).T)

    common = dict(
        x=x, iotaf=iota_f, ident=ident, chemT=chemT, tgtT=tgtT, cellT=cellT
    )
    for k, v in params.items():
        v = np.asarray(v, np.float32)
        if v.ndim == 1:
            v = v[:, None]
        common[k] = np.ascontiguousarray(v)

    in_maps = []
    for c in range(NRANKS):
        m = dict(common)
        m["eidx"] = np.ascontiguousarray(eidx[c])
        m["edst"] = np.ascontiguousarray(edst[c])
        m["enrm"] = np.ascontiguousarray(enrm[c])
        in_maps.append(m)

    global _LAST_IN_MAPS
    _LAST_IN_MAPS = in_maps
    res = run_bass_kernel_spmd(nc, in_maps, core_ids=list(range(NRANKS)))
    outs = res.results

    x_drug = np.concatenate([outs[c]["out_drug"] for c in range(NRANKS)], axis=0)
    x_chem = np.ascontiguousarray(outs[0]["out_chem"].T)
    x_tgt = np.ascontiguousarray(outs[0]["out_tgt"].T)
    x_cell = np.ascontiguousarray(outs[0]["out_cell"].T)
    return (x_drug, x_chem, x_tgt, x_cell)
